# revision 4
# baseline (speedup 1.0000x reference)
"""Multi-head attention on 8 Trainium2 NeuronCores (Bass/Tile) — v2.

Problem: B=4, S=2048, d_model=1024, 16 heads x 64. Full (unsharded) inputs
in, full output out.

Sharding: core c handles batch b=c//2 and head-group g=c%2 (8 of 16 heads);
the output projection is row-sharded and the pair-sum is done on the host
during unsharding (out[b] = part[2b] + part[2b+1] + (bv@Wo + bo), since bv
passes through attention unchanged).

v2 vs v1 (405us): attacks exposed LDWEIGHTS (~77us) and PE idle (~51us).
  - Phase 1: V projection FIRST (attention needs all of V'); Q/K are
    weight-stationary — each 128x128 weight block is loaded once and
    streamed over all 4 token chunks (8x fewer weight loads). x^T chunk
    DMAs spread over idle engine queues.
  - Phase 2: scores for ktile t+1 are emitted BEFORE the PV matmuls of
    ktile t, so when a PV waits on its exp the PE always has a runnable
    matmul next and the 64-deep reorder window pre-pulls the following
    LDWEIGHTS (they were exposed ~150ns/ktile in v1). PSUM: scores 2 bufs
    (4 banks) + PV accumulators 2 bufs (4 banks). The PV tail
    (den->recip->bcast->normalize) reads PV results directly from PSUM
    (no evacuation copy) and is deferred one iteration so its dependency
    chain never blocks the engine FIFOs; double-buffered PV PSUM means
    the next iteration starts with zero stall.
  - Phase 3: output projection with each ot block loaded once (streamed
    over both 512-wide output halves), deep-pipelined PSUM (4 bufs).
  - exp split ACT/DVE 8/8 (ACT table exp / DVE Schraudolph int16 bitcast
    bf16); gpsimd does ONLY partition_broadcast.
"""
import numpy as np
import ml_dtypes

import concourse.bass as bass
import concourse.tile as tile
from concourse import bacc, mybir
from concourse.bass_utils import run_bass_kernel_spmd

F32 = mybir.dt.float32
BF16 = mybir.dt.bfloat16
I16 = mybir.dt.int16
AF = mybir.ActivationFunctionType
ALU = mybir.AluOpType

_S = 2048
_NC_CACHE = {}

# exp engine per ktile: 'A' = ScalarE table exp, 'V' = VectorE Schraudolph.
_EXP_KT = "AVAVAVAVAVAVAVAV"
_SCH_A = 1.4426950408889634 / 8.0 * 128.0
_SCH_B = 127.0 * 128.0 - 366393.0 / 65536.0 + 0.5


def _build(S=_S):
    DM, DQ, H = 1024, 512, 8
    KB, MB = DM // 128, DQ // 128   # 8, 4
    KT, QC, NCH = S // 128, 512, S // 512
    HB = 4                           # head pairs per core

    nc = bacc.Bacc()
    xt_d = nc.declare_dram_parameter("xt", [DM, S], BF16, isOutput=False)
    wq = nc.declare_dram_parameter("wq", [DM, DQ], BF16, isOutput=False)
    wk = nc.declare_dram_parameter("wk", [DM, DQ], BF16, isOutput=False)
    wv = nc.declare_dram_parameter("wv", [DM, DQ], BF16, isOutput=False)
    wo = nc.declare_dram_parameter("wo", [DQ, DM], BF16, isOutput=False)
    bq_pk = nc.declare_dram_parameter("bq_pk", [128, MB], F32, isOutput=False)
    bk_pk = nc.declare_dram_parameter("bk_pk", [128, MB], F32, isOutput=False)
    mv_pk = nc.declare_dram_parameter("mv_pk", [128, KT], F32, isOutput=False)
    out = nc.declare_dram_parameter("out", [S, DM], BF16, isOutput=True)

    with tile.TileContext(nc) as tc:
        with tc.tile_pool(name="persist", bufs=1) as pp:
            bq_sb = pp.tile([128, MB], F32, tag="bq")
            bk_sb = pp.tile([128, MB], F32, tag="bk")
            mv_sb = pp.tile([128, KT], F32, tag="mv")
            mv_bf = pp.tile([128, KT], BF16, tag="mvbf")
            nc.sync.dma_start(bq_sb, bq_pk[:])
            nc.sync.dma_start(bk_sb, bk_pk[:])
            nc.sync.dma_start(mv_sb, mv_pk[:])
            nc.vector.tensor_copy(out=mv_bf, in_=mv_sb)

            # x^T resident [dm-part, kb, S]; weights on separate queues
            # first, then the 4 x^T chunks spread across idle queues.
            xt = pp.tile([128, KB, S], BF16, tag="xt")
            w_eng = {"q": nc.scalar, "k": nc.gpsimd, "v": nc.gpsimd}
            xt_eng = [nc.sync, nc.scalar, nc.gpsimd, nc.sync]
            for n in range(NCH):
                ns = slice(n * QC, (n + 1) * QC)
                xt_eng[n].dma_start(
                    xt[:, :, ns],
                    xt_d.ap()[:, ns].rearrange("(kb p) s -> p kb s", p=128))

            qt = pp.tile([128, MB, S], BF16, tag="qt")
            kt_sb = pp.tile([128, MB, S], BF16, tag="kt")
            # V' [keys, kt, h, 66]: per head 64 v-dims*mask + mask col (65th)
            vp = pp.tile([128, KT, H, 66], BF16, tag="vp")
            ot = pp.tile([128, MB, S], BF16, tag="ot")

            # ---------------- Phase 1: QKV projections -------------------
            with tc.tile_pool(name="wpool", bufs=1) as wp:
                w_r = {}
                for name, w_h in (("q", wq), ("k", wk), ("v", wv)):
                    w_r[name] = wp.tile([128, KB, DQ], BF16,
                                        tag=f"w{name}", name=f"w{name}")
                    w_eng[name].dma_start(
                        w_r[name],
                        w_h.ap().rearrange("(kb p) n -> p kb n", p=128))

                # V first: attention needs the complete V'.
                with nc.named_scope("ph1_v"):
                    with tc.tile_pool(name="vps", bufs=2,
                                      space="PSUM") as vps:
                        for ktile in range(KT):
                            row = slice(ktile * 128, (ktile + 1) * 128)
                            pv = vps.tile([128, DQ], F32, tag="pv")
                            for dj in range(KB):
                                nc.tensor.matmul(
                                    pv, xt[:, dj, row], w_r["v"][:, dj, :],
                                    start=(dj == 0), stop=(dj == KB - 1))
                            nc.vector.tensor_scalar_mul(
                                out=vp[:, ktile, :, 0:64],
                                in0=pv.rearrange("p (h d) -> p h d", h=H),
                                scalar1=mv_sb[:, ktile:ktile + 1])
                            nc.vector.tensor_copy(
                                out=vp[:, ktile, :, 64:65],
                                in_=mv_bf[:, ktile:ktile + 1,
                                          None].to_broadcast((128, H, 1)))

                # Q/K weight-stationary: one weight load per (proj, m, dj),
                # streamed over all 4 token chunks.
                with nc.named_scope("ph1_qk"):
                    with tc.tile_pool(name="qkps", bufs=2,
                                      space="PSUM") as qps:
                        for m in range(MB):
                            mcol = slice(m * 128, (m + 1) * 128)
                            for name, dst, bias in (("q", qt, bq_sb),
                                                    ("k", kt_sb, bk_sb)):
                                p4 = qps.tile([128, NCH, QC], F32,
                                              tag="p4")
                                for dj in range(KB):
                                    for n in range(NCH):
                                        nc.tensor.matmul(
                                            p4[:, n, :],
                                            w_r[name][:, dj, mcol],
                                            xt[:, dj, n * QC:(n + 1) * QC],
                                            start=(dj == 0),
                                            stop=(dj == KB - 1))
                                for n in range(NCH):
                                    ns = slice(n * QC, (n + 1) * QC)
                                    if n % 2 == 0:
                                        nc.scalar.add(
                                            dst[:, m, ns], p4[:, n, :],
                                            bias[:, m:m + 1])
                                    else:
                                        nc.vector.tensor_scalar_add(
                                            out=dst[:, m, ns],
                                            in0=p4[:, n, :],
                                            scalar1=bias[:, m:m + 1])

            # Prefetch Wo while attention runs.
            wo_r = pp.tile([128, MB, DM], BF16, tag="wo")
            nc.sync.dma_start(
                wo_r, wo.ap().rearrange("(m p) n -> p m n", p=128))

            # ---------------- Phase 2: attention -------------------------
            with nc.named_scope("attn"), (
                tc.tile_pool(name="ppool", bufs=1)) as ap2, (
                tc.tile_pool(name="tpool", bufs=1)) as tp, (
                tc.tile_pool(name="scps", bufs=1, space="PSUM")) as sps, (
                tc.tile_pool(name="pvps", bufs=1, space="PSUM")) as ops:

                def tail(pvt, hb, qs):
                    # den (PSUM row 64) -> SBUF -> recip -> gpsimd bcast ->
                    # normalize (numerators read straight from PSUM).
                    den0 = tp.tile([1, 2, QC], F32, tag="den0", bufs=2,
                                   name="den0")
                    nc.scalar.copy(den0, pvt[64:65, :, :])
                    denr = tp.tile([1, 2, QC], F32, tag="denr", bufs=2,
                                   name="denr")
                    nc.vector.reciprocal_approx_fast(out=denr, in_=den0)
                    repd = tp.tile([64, 2, QC], F32, tag="repd", bufs=2,
                                   name="repd")
                    nc.gpsimd.partition_broadcast(
                        repd[:, 0, :], denr[0:1, 0, :], channels=64)
                    nc.gpsimd.partition_broadcast(
                        repd[:, 1, :], denr[0:1, 1, :], channels=64)
                    nc.vector.tensor_mul(
                        out=ot[0:64, hb, qs],
                        in0=pvt[0:64, 0, :], in1=repd[:, 0, :])
                    shf = tp.tile([64, QC], BF16, tag="shf", bufs=2,
                                  name="shf")
                    nc.vector.tensor_mul(
                        out=shf, in0=pvt[0:64, 1, :], in1=repd[:, 1, :])
                    nc.sync.dma_start(ot[64:128, hb, qs], shf)

                def scores_exp(t, hb, qs):
                    ks = slice(t * 128, (t + 1) * 128)
                    sb = sps.tile([128, 2, QC], F32, tag="sb", bufs=2)
                    nc.tensor.matmul(
                        sb[:, 0, :], kt_sb[0:64, hb, ks],
                        qt[0:64, hb, qs], start=True, stop=True)
                    nc.tensor.matmul(
                        sb[:, 1, :], kt_sb[64:128, hb, ks],
                        qt[64:128, hb, qs], start=True, stop=True)
                    pb = ap2.tile([128, 2, QC], BF16, tag="pb", bufs=4)
                    if _EXP_KT[t] == "A":
                        nc.scalar.activation(pb, sb, AF.Exp, scale=0.125)
                    else:
                        nc.vector.tensor_scalar(
                            out=pb.bitcast(I16), in0=sb,
                            scalar1=_SCH_A, scalar2=_SCH_B,
                            op0=ALU.mult, op1=ALU.add)
                    return pb

                pending = None
                for it in range(NCH * HB):
                    q, hb = it // HB, it % HB
                    h0, h1 = 2 * hb, 2 * hb + 1
                    qs = slice(q * QC, (q + 1) * QC)
                    if pending is not None:
                        tail(*pending)
                        pending = None
                    pvt = ops.tile([128, 2, QC], F32, tag="pv", bufs=2)
                    pb = scores_exp(0, hb, qs)
                    for t in range(KT):
                        pb_next = (scores_exp(t + 1, hb, qs)
                                   if t + 1 < KT else None)
                        nc.tensor.matmul(
                            pvt[0:65, 0, :], vp[:, t, h0, 0:65],
                            pb[:, 0, :],
                            start=(t == 0), stop=(t == KT - 1))
                        nc.tensor.matmul(
                            pvt[0:65, 1, :], vp[:, t, h1, 0:65],
                            pb[:, 1, :],
                            start=(t == 0), stop=(t == KT - 1))
                        pb = pb_next
                    pending = (pvt, hb, qs)
                tail(*pending)

            # ---------------- Phase 3: output projection -----------------
            with nc.named_scope("proj"), (
                tc.tile_pool(name="prj", bufs=4)) as prp, (
                tc.tile_pool(name="prps", bufs=4, space="PSUM")) as fps:
                for qt_i in range(S // 128):
                    rows = slice(qt_i * 128, (qt_i + 1) * 128)
                    pf = fps.tile([128, 2, 512], F32, tag="pf")
                    for m in range(MB):
                        for ncb in range(2):
                            nc.tensor.matmul(
                                pf[:, ncb, :],
                                ot[:, m, rows],
                                wo_r[:, m, ncb * 512:(ncb + 1) * 512],
                                start=(m == 0), stop=(m == MB - 1))
                    for ncb in range(2):
                        o_st = prp.tile([128, 512], BF16, tag="ost")
                        if ncb == 0:
                            nc.vector.tensor_copy(out=o_st, in_=pf[:, 0, :])
                        else:
                            nc.scalar.copy(o_st, pf[:, 1, :])
                        nc.sync.dma_start(
                            out.ap()[rows, ncb * 512:(ncb + 1) * 512],
                            o_st)

    nc.compile()
    return nc


def get_nc(S=_S):
    if S not in _NC_CACHE:
        _NC_CACHE[S] = _build(S)
    return _NC_CACHE[S]


def _bf16(a):
    return np.ascontiguousarray(a.astype(ml_dtypes.bfloat16))


def shard_inputs(inputs, S=_S):
    x = np.asarray(inputs["x"], dtype=np.float32)
    mask = np.asarray(inputs["attention_mask"])
    Wq, Wk, Wv, Wo = (np.asarray(inputs[k], dtype=np.float32)
                      for k in ("Wq", "Wk", "Wv", "Wo"))
    bq, bk, bv, bo = (np.asarray(inputs[k], dtype=np.float32)
                      for k in ("bq", "bk", "bv", "bo"))
    in_maps = []
    for c in range(8):
        b, g = c // 2, c % 2
        cols = slice(g * 512, (g + 1) * 512)
        in_maps.append({
            "xt": _bf16(x[b, :S].T),
            "wq": _bf16(Wq[:, cols]),
            "wk": _bf16(Wk[:, cols]),
            "wv": _bf16(Wv[:, cols]),
            "wo": _bf16(Wo[cols, :]),
            "bq_pk": np.ascontiguousarray(bq[cols].reshape(4, 128).T),
            "bk_pk": np.ascontiguousarray(bk[cols].reshape(4, 128).T),
            "mv_pk": np.ascontiguousarray(
                mask[b, :S].astype(np.float32).reshape(S // 128, 128).T),
        })
    host_bias = bv @ Wo + bo   # bv passes through attention unchanged
    return in_maps, host_bias


def unshard_outputs(results, host_bias, S=_S):
    out = np.empty((4, S, 1024), dtype=np.float32)
    for b in range(4):
        out[b] = (np.asarray(results[2 * b]["out"], dtype=np.float32)
                  + np.asarray(results[2 * b + 1]["out"], dtype=np.float32)
                  + host_bias)
    return out


def kernel(**inputs):
    nc = get_nc()
    in_maps, host_bias = shard_inputs(inputs)
    res = run_bass_kernel_spmd(nc, in_maps, core_ids=list(range(8)))
    return unshard_outputs(res.results, host_bias)
